# revision 55
# baseline (speedup 1.0000x reference)
"""Distributed Trainium2 kernel for ArceeAttention (GQA + RoPE + causal attention).

Sharding: DP over batch (2 groups of 4 cores) x TP-4 over heads within each
group. Each core: 8 q heads + 2 kv heads, full sequence of its batch.
ReduceScatter(add) over each 4-core group combines o-proj partials per
512-token slab, split into two half-RS so communication overlaps compute.

Design (v2, ~834us in TimelineSim vs 1045us baseline; HW-verified 7.0e-3):
  - Host side: hidden/w_qkv/w_o cast to bf16, RoPE cos/sin precomputed as
    bf16 [128, S] tables, causal masks + sign-baked rotate-half swap matrix.
    No SWDGE casts, no on-device trig, no DRAM staging round trips.
  - A0 (QKV+RoPE), CHUNK=512: hidden transposed straight from DRAM via XBAR
    in [512,512]->[128,4,512] grouped transposes, ALL on the SP queue (two
    concurrent XBAR transposes on different HWDGE queues corrupt each other
    on HW). Weights loaded as 4 COLUMN-block DMAs on the Act queue; chunk 0
    runs h-major in per-block groups of <=3 ct chains, so each ~9us block
    transfer feeds ~20us of PE work and the pipeline never starves after
    block 0 lands. RoPE is pipelined one ct behind its QKV chain;
    rotate-half via a sign-baked bf16 swap matmul.
  - qT spilled to per-chunk DRAM tiles (frees SBUF for the 96KB weight
    tile), reloaded per slab in A1 with one strided DMA; kT/V stay resident.
  - A1: causal attention with 512-wide transposed score blocks, exp without
    max-subtraction, denominator via ones-matmul + fast reciprocal; PV
    accumulation pipelined one kt behind the scores/exp. O-proj of slab s-1
    is interleaved between attention heads of slab s so the PE fills the
    Act-engine exp latency; outputs staged as [128, HID] rows (4 DMAs/slab).
  - Output dtype bf16 on device; host assembles/casts to f32.
"""
import sys
import numpy as np

for _p in ("/opt/trn_rl_repo",):
    if _p not in sys.path:
        sys.path.append(_p)

import ml_dtypes  # noqa: E402
from concourse import bass, bacc, tile, mybir  # noqa: E402
from concourse.bass_utils import run_bass_kernel_spmd  # noqa: E402

F32 = mybir.dt.float32
BF16 = mybir.dt.bfloat16
I32 = mybir.dt.int32

ROPE_THETA = 10000.0
D = 128  # head dim

_NC_CACHE = {}
TRACE_HOOK = None


def _lbl(s):
    if TRACE_HOOK:
        TRACE_HOOK(s)


def build_nc(S=2048, HID=4096, NQ=8, NKV=2, CHUNK=512, debug=False):
    REP = NQ // NKV
    QC = NQ * D               # q cols per core
    KC = NKV * D              # k (or v) cols per core
    NCH = S // CHUNK          # token chunks in A0
    KTC = CHUNK // 128
    NHT = HID // 128          # hidden-dim tiles
    SLAB = min(512, S)        # tokens per ReduceScatter slab / query block
    NSLAB = S // SLAB
    SKT = SLAB // 128         # k-tiles per slab
    NHC = HID // 512          # o-proj col chunks
    W = 512
    HG = 2                    # o-proj psum group size
    SCALE = float(D) ** -0.5
    RG = [[0, 1, 2, 3], [4, 5, 6, 7]]

    MAGIC = 12582912.0        # 1.5 * 2**23: float32 round-to-nearest-int trick
    TWOPI = float(2.0 * np.pi)
    INV2PI = float(1.0 / TWOPI)
    HALFPI = float(np.pi / 2.0)

    nc = bacc.Bacc(None, target_bir_lowering=False)
    hidden = nc.declare_dram_parameter("hidden_states", [S, HID], BF16, isOutput=False)
    w_qkv = nc.declare_dram_parameter("w_qkv", [HID, QC + 2 * KC], BF16, isOutput=False)
    w_o = nc.declare_dram_parameter("w_o", [QC, HID], BF16, isOutput=False)
    cosd = nc.declare_dram_parameter("cos_all", [128, S], BF16, isOutput=False)
    sind = nc.declare_dram_parameter("sin_all", [128, S], BF16, isOutput=False)
    masks = nc.declare_dram_parameter("masks", [128, SKT, SLAB], BF16, isOutput=False)
    swapm = nc.declare_dram_parameter("swapmat", [128, 128], BF16, isOutput=False)
    out = nc.declare_dram_parameter("out", [NSLAB, SLAB // 4, HID], BF16, isOutput=True)
    if debug:
        dbg_qT = nc.declare_dram_parameter("dbg_qT", [NQ, 128, S], BF16,
                                           isOutput=True)
        dbg_kT = nc.declare_dram_parameter("dbg_kT", [NKV, 128, S], BF16,
                                           isOutput=True)
        dbg_v = nc.declare_dram_parameter("dbg_v", [S // 128, 128, 2 * D], BF16,
                                          isOutput=True)

    Exp = mybir.ActivationFunctionType.Exp

    with tile.TileContext(nc) as tc:
      with tc.tile_pool(name="dram", bufs=1, space="DRAM") as dram:
        qT_dram = [dram.tile([NQ, 128, CHUNK], BF16, name=f"qT_dram{c}",
                             tag=f"qT_dram{c}") for c in range(NCH)]
        parts = [dram.tile([SLAB, HID], BF16, name=f"part{s}", tag=f"part{s}")
                 for s in range(NSLAB)]
        rsouts = [dram.tile([SLAB // 4, HID], BF16, name=f"rsout{s}", tag=f"rsout{s}")
                  for s in range(NSLAB)]

        with tc.tile_pool(name="const", bufs=1) as cpool:
            cos_sb = cpool.tile([128, S], BF16, name="cos_sb", tag="cos_sb")
            sin_sb = cpool.tile([128, S], BF16, name="sin_sb", tag="sin_sb")
            ones_col = cpool.tile([128, 1], BF16, name="ones_col", tag="ones_col")
            nc.vector.memset(ones_col[:], 1.0)
            ones_row = cpool.tile([1, 128], F32, name="ones_row", tag="ones_row")
            nc.vector.memset(ones_row[:], 1.0)
            ones_row_bf = cpool.tile([1, 128], BF16, name="ones_row_bf",
                                     tag="ones_row_bf")
            nc.vector.memset(ones_row_bf[:], 1.0)
            swap_sb = cpool.tile([128, 128], BF16, name="swapm", tag="swapm")
            mask_sb = cpool.tile([128, SKT, SLAB], BF16, name="masks",
                                 tag="masks")

            # persistent across A0 -> A1
            with tc.tile_pool(name="qkv_keep", bufs=1) as kvp:
                kT_sb = [kvp.tile([128, S], BF16, name=f"kT{i}", tag=f"kT{i}")
                         for i in range(NKV)]
                v_sb = [kvp.tile([128, KC], BF16, name=f"v{t}", tag=f"v{t}")
                        for t in range(S // 128)]

                # ================= A0: QKV + RoPE =================
                with (
                    tc.tile_pool(name="wq", bufs=1) as wqp,
                    tc.tile_pool(name="hidT", bufs=2) as hTp,
                    tc.tile_pool(name="rope", bufs=2) as rp,
                    tc.tile_pool(name="psA", bufs=5, space="PSUM") as psA,
                    tc.tile_pool(name="psT", bufs=2, space="PSUM") as psT,
                ):
                    TGRP = 4   # h-tiles per transpose instruction (3D out)

                    def emit_transposes(c):
                        _lbl(f"A0.trans.c{c}")
                        groups = [hTp.tile([128, TGRP, CHUNK], BF16,
                                           name=f"hidTg{g}", tag=f"hidTg{g}")
                                  for g in range(NHT // TGRP)]
                        c0 = CHUNK * c
                        for g in range(NHT // TGRP):
                            # all transposes on ONE queue: two concurrent XBAR
                            # transposes (SP+Act) corrupt each other's 16-row
                            # tiles on hardware
                            nc.sync.dma_start_transpose(
                                groups[g][:],
                                hidden[c0:c0 + CHUNK,
                                       128 * TGRP * g:128 * TGRP * (g + 1)],
                            )
                        return groups

                    # startup: interleave wq-group loads (Act queue) with
                    # chunk-0 transpose groups (SP queue) in consumption order
                    # so DMA_ENGINES serves them h-ordered; consts at the end
                    _lbl("A0.wqload")
                    # weight load in 4 COLUMN blocks of 384 qkv-cols: block j
                    # feeds chunk-0 pass j (ct 3j..3j+2; block 3 also ct9+V).
                    # One block transfers in ~9us but feeds ~20us of PE work,
                    # so after block 0 lands the chunk-0 pipeline never stalls.
                    CB = (QC + 2 * KC) // 4
                    wq_b = [wqp.tile([128, NHT, CB], BF16,
                                     name=f"wqb{j}", tag=f"wqb{j}")
                            for j in range(4)]
                    hidT = emit_transposes(0)
                    for j in range(4):
                        nc.scalar.dma_start(
                            wq_b[j][:],
                            w_qkv[:, CB * j:CB * (j + 1)]
                            .rearrange("(h p) c -> p h c", p=128),
                        )
                    nc.sync.dma_start(cos_sb[:], cosd[:])
                    nc.sync.dma_start(sin_sb[:], sind[:])
                    nc.sync.dma_start(swap_sb[:], swapm[:])
                    nc.sync.dma_start(mask_sb[:], masks[:])

                    def wq_ap(h, lo, hi):
                        j = lo // CB
                        assert hi <= CB * (j + 1)
                        return wq_b[j][:, h, lo - CB * j:hi - CB * j]


                    for c in range(NCH):
                        c0 = CHUNK * c
                        # chunk 1's transposes are deferred to after chunk 0's
                        # ct loop: at startup the DMA sem lanes are saturated
                        # and they'd crowd out the wq group loads
                        hidT_next = (emit_transposes(c + 1)
                                     if 1 < c + 1 < NCH else None)
                        cosc = cos_sb[:, c0:c0 + CHUNK]
                        sinc = sin_sb[:, c0:c0 + CHUNK]
                        # Q^T / K^T col-tiles; RoPE pipelined one ct behind
                        # so pswap(ct) never stalls the PE behind qf(ct)
                        def emit_rope(ct, qf, c=c, c0=c0, cosc=cosc, sinc=sinc):
                            _lbl(f"A0.c{c}.rope{ct}")
                            pswap = psT.tile([128, CHUNK], F32, name="pswap",
                                             tag="pswap")
                            nc.tensor.matmul(pswap[:], swap_sb[:], qf[:],
                                             start=True, stop=True)
                            rot = rp.tile([128, CHUNK], BF16, name="rot", tag="rot")
                            nc.vector.tensor_mul(rot[:], pswap[:], sinc)
                            tc2 = rp.tile([128, CHUNK], BF16, name="tc2", tag="tc2")
                            nc.vector.tensor_mul(tc2[:], qf[:], cosc)
                            if ct < NQ:
                                qs = rp.tile([128, CHUNK], BF16, name="qs",
                                             tag="qs")
                                nc.vector.tensor_add(qs[:], tc2[:], rot[:])
                                nc.gpsimd.dma_start(
                                    qT_dram[c][ct, :, :], qs[:])
                            else:
                                nc.vector.tensor_add(
                                    kT_sb[ct - NQ][:, c0:c0 + CHUNK],
                                    tc2[:], rot[:])

                        pending = None
                        if c == 0:
                            # chunk 0: h-major in groups of <=3 ct chains so the
                            # PE consumes wq tiles as their DMAs land instead of
                            # stalling until the full weight load completes
                            groups = [(0, 1, 2), (3, 4, 5), (6, 7, 8), (9,)]
                            for grp in groups:
                                _lbl(f"A0.c0.grp{grp[0]}")
                                pq_map = {
                                    ct: psA.tile([128, CHUNK], F32,
                                                 name="pq", tag="pq")
                                    for ct in grp
                                }
                                for h in range(NHT):
                                    for ct in grp:
                                        nc.tensor.matmul(
                                            pq_map[ct][:],
                                            wq_ap(h, 128 * ct, 128 * (ct + 1)),
                                            hidT[h // TGRP][:, h % TGRP, :],
                                            start=(h == 0), stop=(h == NHT - 1),
                                        )
                                for ct in grp:
                                    qf = rp.tile([128, CHUNK], BF16,
                                                 name="qf", tag="qf")
                                    nc.scalar.copy(qf[:], pq_map[ct][:])
                                    if pending is not None:
                                        emit_rope(*pending)
                                    pending = (ct, qf)
                        else:
                            for ct in range(NQ + NKV):
                                _lbl(f"A0.c{c}.ct{ct}")
                                pq = psA.tile([128, CHUNK], F32,
                                              name="pq", tag="pq")
                                for h in range(NHT):
                                    nc.tensor.matmul(
                                        pq[:],
                                        wq_ap(h, 128 * ct, 128 * (ct + 1)),
                                        hidT[h // TGRP][:, h % TGRP, :],
                                        start=(h == 0), stop=(h == NHT - 1),
                                    )
                                qf = rp.tile([128, CHUNK], BF16,
                                             name="qf", tag="qf")
                                nc.scalar.copy(qf[:], pq[:])
                                if pending is not None:
                                    emit_rope(*pending)
                                pending = (ct, qf)
                        emit_rope(*pending)
                        if c == 0 and NCH > 1:
                            hidT_next = emit_transposes(1)
                        # V natural [tok, d]
                        for tt in range(KTC):
                            _lbl(f"A0.c{c}.v{tt}")
                            pv = psA.tile([128, KC], F32, name="pv", tag="pq")
                            for h in range(NHT):
                                nc.tensor.matmul(
                                    pv[:],
                                    hidT[h // TGRP][:, h % TGRP,
                                         128 * tt:128 * (tt + 1)],
                                    wq_ap(h, QC + KC, QC + 2 * KC),
                                    start=(h == 0), stop=(h == NHT - 1),
                                )
                            nc.scalar.copy(v_sb[c * KTC + tt][:], pv[:])
                        hidT = hidT_next

                if debug:
                    _lbl("debug.dump")
                    for c in range(NCH):
                        nc.sync.dma_start(
                            dbg_qT[:, :, CHUNK * c:CHUNK * (c + 1)],
                            qT_dram[c][:])
                    for i in range(NKV):
                        nc.sync.dma_start(dbg_kT[i], kT_sb[i][:])
                    for t in range(S // 128):
                        nc.sync.dma_start(dbg_v[t], v_sb[t][:])

                # ============ A1: attention + o-proj + RS ============
                with (
                    tc.tile_pool(name="wo", bufs=1) as wop,
                    tc.tile_pool(name="qsl", bufs=2) as qslp,
                    tc.tile_pool(name="at", bufs=2) as atp,
                    tc.tile_pool(name="pt", bufs=6) as ptp,
                    tc.tile_pool(name="den", bufs=2) as dnp,
                    tc.tile_pool(name="bcp", bufs=2) as bcp,
                    tc.tile_pool(name="ost", bufs=2) as ostp,
                    tc.tile_pool(name="psS", bufs=2, space="PSUM") as psS,
                    tc.tile_pool(name="psPV", bufs=2, space="PSUM") as psPV,
                    tc.tile_pool(name="psX", bufs=1, space="PSUM") as psX,
                    tc.tile_pool(name="psO", bufs=3, space="PSUM") as psO,
                ):
                    _lbl("A1.load")

                    def load_qsl(s):
                        # SLAB == CHUNK: slab s reads exactly chunk s's spill
                        qsl = qslp.tile([128, NQ, SLAB], BF16, name="qsl",
                                        tag="qsl")
                        nc.sync.dma_start(
                            qsl[:],
                            qT_dram[s][:].rearrange("h p c -> p h c"),
                        )
                        return qsl

                    qsl = load_qsl(0)
                    wo_sb = wop.tile([128, NQ, HID], BF16, name="wo", tag="wo")
                    CQ = HID // 4
                    for i in range(4):
                        eng = nc.sync if i % 2 == 0 else nc.scalar
                        eng.dma_start(
                            wo_sb[:, :, CQ * i:CQ * (i + 1)],
                            w_o[:, CQ * i:CQ * (i + 1)]
                            .rearrange("(q p) c -> p q c", p=128),
                        )
                    NHG = NHC // HG          # hg groups per tt (4)
                    NGRP = (SLAB // 128) * NHG  # o-proj groups per slab (16)

                    def make_oproj_emitter(s, at_tiles):
                        state = {}

                        def emit_groups(glo, ghi):
                            for g in range(glo, ghi):
                                tt, hg = divmod(g, NHG)
                                _lbl(f"A1.s{s}.oproj{tt}.{hg}")
                                if hg == 0:
                                    state["ost"] = ostp.tile(
                                        [128, HID], BF16, name="ost", tag="ost")
                                ost = state["ost"]
                                pos = [psO.tile([128, W], F32,
                                                name=f"po{j}", tag="po")
                                       for j in range(HG)]
                                for q in range(NQ):
                                    for j in range(HG):
                                        hc = hg * HG + j
                                        nc.tensor.matmul(
                                            pos[j][:],
                                            at_tiles[q][:, 128 * tt:128 * (tt + 1)],
                                            wo_sb[:, q, W * hc:W * (hc + 1)],
                                            start=(q == 0), stop=(q == NQ - 1),
                                        )
                                for j in range(HG):
                                    hc = hg * HG + j
                                    nc.vector.tensor_copy(
                                        ost[:, W * hc:W * (hc + 1)], pos[j][:])
                                if hg == NHG - 1:
                                    nc.sync.dma_start(
                                        parts[s][128 * tt:128 * (tt + 1), :],
                                        ost[:])
                        return emit_groups

                    def emit_rs(s, half):
                        _lbl(f"A1.s{s}.rs{half}")
                        r0 = (SLAB // 2) * half
                        o0 = (SLAB // 8) * half
                        nc.gpsimd.collective_compute(
                            "ReduceScatter",
                            mybir.AluOpType.add,
                            replica_groups=RG,
                            ins=[parts[s][r0:r0 + SLAB // 2, :]],
                            outs=[rsouts[s][o0:o0 + SLAB // 8, :]],
                        )
                        nc.gpsimd.dma_start(
                            out[s, o0:o0 + SLAB // 8, :],
                            rsouts[s][o0:o0 + SLAB // 8, :])

                    prev_emit = None  # o-proj emitter for slab s-1
                    for s in range(NSLAB):
                        s0 = SLAB * s
                        NKT = (s + 1) * SKT
                        qsl_next = load_qsl(s + 1) if s + 1 < NSLAB else None
                        at_tiles = []
                        for hq in range(NQ):
                            _lbl(f"A1.s{s}.h{hq}")
                            kvh = hq // REP
                            ppv = psPV.tile([128, SLAB], F32, name="ppv",
                                            tag="ppv")
                            den = dnp.tile([128, SLAB], BF16, name="den",
                                           tag="den")
                            # PV pipelined one kt behind: ps(kt+1) is emitted
                            # before ppv(kt) so the PE never waits on the exp
                            # round trip
                            pend_pv = None
                            for kt in range(NKT):
                                ps = psS.tile([128, SLAB], F32, name="ps",
                                              tag="ps")
                                nc.tensor.matmul(
                                    ps[:],
                                    kT_sb[kvh][:, 128 * kt:128 * (kt + 1)],
                                    qsl[:, hq, :],
                                    start=True, stop=True,
                                )
                                pt = ptp.tile([128, SLAB], BF16, name="pt",
                                              tag="pt")
                                nc.scalar.activation(pt[:], ps[:], Exp,
                                                     scale=SCALE)
                                diag = kt - s * SKT
                                if diag >= 0:
                                    nc.vector.tensor_mul(
                                        pt[:], pt[:], mask_sb[:, diag, :])
                                if kt == 0:
                                    nc.vector.tensor_copy(den[:], pt[:])
                                else:
                                    nc.vector.tensor_add(den[:], den[:], pt[:])
                                if pend_pv is not None:
                                    pkt, ppt = pend_pv
                                    nc.tensor.matmul(
                                        ppv[:],
                                        v_sb[pkt][:, D * kvh:D * (kvh + 1)],
                                        ppt[:],
                                        start=(pkt == 0), stop=False,
                                    )
                                pend_pv = (kt, pt)
                            pkt, ppt = pend_pv
                            nc.tensor.matmul(
                                ppv[:],
                                v_sb[pkt][:, D * kvh:D * (kvh + 1)],
                                ppt[:],
                                start=(pkt == 0), stop=True,
                            )
                            pden = psX.tile([128, SLAB], F32, name="pden",
                                            tag="pden")
                            nc.tensor.matmul(pden[0:1, :], ones_col[:], den[:],
                                             start=True, stop=True)
                            rec = dnp.tile([1, SLAB], F32, name="rec", tag="rec")
                            nc.vector.reciprocal_approx_fast(rec[:], pden[0:1, :])
                            recb = dnp.tile([1, SLAB], BF16, name="recb",
                                            tag="recb")
                            nc.vector.tensor_copy(recb[:], rec[:])
                            pbc = psX.tile([128, SLAB], F32, name="pbc",
                                           tag="pden")
                            nc.tensor.matmul(pbc[:], ones_row_bf[:], recb[:],
                                             start=True, stop=True)
                            bc = bcp.tile([128, SLAB], F32, name="bc", tag="bc")
                            nc.vector.tensor_copy(bc[:], pbc[:])
                            at = atp.tile([128, SLAB], BF16,
                                          name=f"at{hq}", tag=f"at{hq}")
                            nc.vector.tensor_mul(at[:], ppv[:], bc[:])
                            at_tiles.append(at)
                            # interleave previous slab's o-proj between heads
                            if prev_emit is not None:
                                prev_emit(2 * hq, 2 * hq + 2)
                                if hq == 3:
                                    emit_rs(s - 1, 0)
                                elif hq == 7:
                                    emit_rs(s - 1, 1)
                        prev_emit = make_oproj_emitter(s, at_tiles)
                        qsl = qsl_next
                    # final slab's o-proj + RS
                    prev_emit(0, NGRP // 2)
                    emit_rs(NSLAB - 1, 0)
                    prev_emit(NGRP // 2, NGRP)
                    emit_rs(NSLAB - 1, 1)

    nc.compile()
    return nc


def make_consts(S=2048):
    SLAB = min(512, S)
    SKT = SLAB // 128
    p = np.arange(128).reshape(128, 1, 1)
    j = np.arange(SKT).reshape(1, SKT, 1)
    q = np.arange(SLAB).reshape(1, 1, SLAB)
    masks = ((j * 128 + p) <= q).astype(ml_dtypes.bfloat16)  # [128, SKT, SLAB]
    # swap matrix with RoPE sign baked in: out[m<64] = -x[m+64], out[m>=64] = +x[m-64]
    swapmat = np.zeros((128, 128), np.float32)
    for m in range(128):
        swapmat[(m + 64) % 128, m] = 1.0 if m >= 64 else -1.0
    return masks, swapmat.astype(ml_dtypes.bfloat16)


def host_trig(positions_row):
    """cos/sin [128, S] bf16 for one batch's positions (RoPE angles)."""
    d_half = np.arange(0, D, 2, dtype=np.float64) / D
    invf = 1.0 / (ROPE_THETA ** d_half)                 # [64]
    invf128 = np.concatenate([invf, invf])              # [128]
    ang = invf128[:, None] * positions_row[None, :].astype(np.float64)
    return (np.cos(ang).astype(ml_dtypes.bfloat16),
            np.sin(ang).astype(ml_dtypes.bfloat16))


def shard_inputs(hidden_states, positions, w_qkv, w_o, n_q_total=32, n_kv_total=8,
                 tp=4):
    """Returns in_maps for 8 cores: DP over batch x TP over heads."""
    B, S, HIDDEN = hidden_states.shape
    q_size = n_q_total * D
    kv_size = n_kv_total * D
    nq = n_q_total // tp           # q heads per core
    nkv = n_kv_total // tp         # kv heads per core
    masks, swapmat = make_consts(S=S)
    hid_bf = hidden_states.astype(ml_dtypes.bfloat16)
    trigs = [host_trig(np.asarray(positions[g]).astype(np.int64))
             for g in range(B)]
    wq_bf = w_qkv.astype(ml_dtypes.bfloat16)
    wo_bf = w_o.astype(ml_dtypes.bfloat16)
    in_maps = []
    for c in range(8):
        g, r = divmod(c, tp)
        wq = wq_bf[:, nq * D * r: nq * D * (r + 1)]
        wk = wq_bf[:, q_size + nkv * D * r: q_size + nkv * D * (r + 1)]
        wv = wq_bf[:, q_size + kv_size + nkv * D * r:
                   q_size + kv_size + nkv * D * (r + 1)]
        in_maps.append({
            "hidden_states": np.ascontiguousarray(hid_bf[g]),
            "w_qkv": np.ascontiguousarray(np.concatenate([wq, wk, wv], axis=1)),
            "w_o": np.ascontiguousarray(wo_bf[nq * D * r: nq * D * (r + 1), :]),
            "cos_all": trigs[g][0],
            "sin_all": trigs[g][1],
            "masks": masks,
            "swapmat": swapmat,
        })
    return in_maps


def assemble_output(results, B=2, S=2048, HIDDEN=4096, tp=4):
    SLAB = min(512, S)
    NSLAB = S // SLAB
    RSROWS = SLAB // 4   # rows per core per slab (128)
    RSH = SLAB // 8      # rows per core per half-RS (64)
    out = np.empty((B, S, HIDDEN), dtype=np.float32)
    for c in range(8):
        g, r = divmod(c, tp)
        o = np.asarray(results[c]["out"]).astype(np.float32).reshape(
            NSLAB, RSROWS, HIDDEN)
        for s in range(NSLAB):
            for h in range(2):
                t0 = SLAB * s + (SLAB // 2) * h + RSH * r
                out[g, t0:t0 + RSH, :] = o[s, RSH * h:RSH * (h + 1), :]
    return out


def kernel(hidden_states, positions, w_qkv, w_o):
    hidden_states = np.asarray(hidden_states, dtype=np.float32)
    positions = np.asarray(positions, dtype=np.int32)
    w_qkv = np.asarray(w_qkv, dtype=np.float32)
    w_o = np.asarray(w_o, dtype=np.float32)
    B, S, HIDDEN = hidden_states.shape

    key = (S, HIDDEN)
    if key not in _NC_CACHE:
        _NC_CACHE[key] = build_nc(S=S, HID=HIDDEN)
    nc = _NC_CACHE[key]

    in_maps = shard_inputs(hidden_states, positions, w_qkv, w_o)
    res = run_bass_kernel_spmd(nc, in_maps, core_ids=list(range(8)))
    return assemble_output(res.results, B=B, S=S, HIDDEN=HIDDEN)


if __name__ == "__main__":
    rng = np.random.default_rng(0)
    B, S, HIDDEN = 2, 2048, 4096
    hs = rng.standard_normal((B, S, HIDDEN), dtype=np.float32)
    pos = np.arange(B * S, dtype=np.int32).reshape(B, S)
    wq = rng.standard_normal((HIDDEN, 6144), dtype=np.float32) * HIDDEN ** -0.5
    wo = rng.standard_normal((4096, HIDDEN), dtype=np.float32) * 4096 ** -0.5
    o = kernel(hs, pos, wq, wo)
    print(o.shape, o.dtype)


# revision 57
# speedup vs baseline: 1.0616x; 1.0616x over previous
"""Distributed Trainium2 kernel for ArceeAttention (GQA + RoPE + causal attention).

Sharding: DP over batch (2 groups of 4 cores) x TP-4 over heads within each
group. Each core: 8 q heads + 2 kv heads, full sequence of its batch.
ReduceScatter(add) over each 4-core group combines o-proj partials per
512-token slab, split into two half-RS so communication overlaps compute.

Design (v2, ~834us in TimelineSim vs 1045us baseline; HW-verified 7.0e-3):
  - Host side: hidden/w_qkv/w_o cast to bf16, RoPE cos/sin precomputed as
    bf16 [128, S] tables, causal masks + sign-baked rotate-half swap matrix.
    No SWDGE casts, no on-device trig, no DRAM staging round trips.
  - A0 (QKV+RoPE), CHUNK=512: hidden transposed straight from DRAM via XBAR
    in [512,512]->[128,4,512] grouped transposes, ALL on the SP queue (two
    concurrent XBAR transposes on different HWDGE queues corrupt each other
    on HW). Weights loaded as 4 COLUMN-block DMAs on the Act queue; chunk 0
    runs h-major in per-block groups of <=3 ct chains, so each ~9us block
    transfer feeds ~20us of PE work and the pipeline never starves after
    block 0 lands. RoPE is pipelined one ct behind its QKV chain;
    rotate-half via a sign-baked bf16 swap matmul.
  - qT spilled to per-chunk DRAM tiles (frees SBUF for the 96KB weight
    tile), reloaded per slab in A1 with one strided DMA; kT/V stay resident.
  - A1: causal attention with 512-wide transposed score blocks, exp without
    max-subtraction, denominator via ones-matmul + fast reciprocal; PV
    accumulation pipelined one kt behind the scores/exp. O-proj of slab s-1
    is interleaved between attention heads of slab s so the PE fills the
    Act-engine exp latency; outputs staged as [128, HID] rows (4 DMAs/slab).
  - Output dtype bf16 on device; host assembles/casts to f32.
"""
import sys
import numpy as np

for _p in ("/opt/trn_rl_repo",):
    if _p not in sys.path:
        sys.path.append(_p)

import ml_dtypes  # noqa: E402
from concourse import bass, bacc, tile, mybir  # noqa: E402
from concourse.bass_utils import run_bass_kernel_spmd  # noqa: E402

F32 = mybir.dt.float32
BF16 = mybir.dt.bfloat16
I32 = mybir.dt.int32

ROPE_THETA = 10000.0
D = 128  # head dim

_NC_CACHE = {}
TRACE_HOOK = None


def _lbl(s):
    if TRACE_HOOK:
        TRACE_HOOK(s)


def build_nc(S=2048, HID=4096, NQ=8, NKV=2, CHUNK=512, debug=False):
    REP = NQ // NKV
    QC = NQ * D               # q cols per core
    KC = NKV * D              # k (or v) cols per core
    NCH = S // CHUNK          # token chunks in A0
    KTC = CHUNK // 128
    NHT = HID // 128          # hidden-dim tiles
    SLAB = min(512, S)        # tokens per ReduceScatter slab / query block
    NSLAB = S // SLAB
    SKT = SLAB // 128         # k-tiles per slab
    NHC = HID // 512          # o-proj col chunks
    W = 512
    HG = 2                    # o-proj psum group size
    SCALE = float(D) ** -0.5
    RG = [[0, 1, 2, 3], [4, 5, 6, 7]]

    MAGIC = 12582912.0        # 1.5 * 2**23: float32 round-to-nearest-int trick
    TWOPI = float(2.0 * np.pi)
    INV2PI = float(1.0 / TWOPI)
    HALFPI = float(np.pi / 2.0)

    nc = bacc.Bacc(None, target_bir_lowering=False)
    hidden = nc.declare_dram_parameter("hidden_states", [S, HID], BF16, isOutput=False)
    w_qkv = nc.declare_dram_parameter("w_qkv", [HID, QC + 2 * KC], BF16, isOutput=False)
    w_o = nc.declare_dram_parameter("w_o", [QC, HID], BF16, isOutput=False)
    cosd = nc.declare_dram_parameter("cos_all", [128, S], BF16, isOutput=False)
    sind = nc.declare_dram_parameter("sin_all", [128, S], BF16, isOutput=False)
    masks = nc.declare_dram_parameter("masks", [128, SKT, SLAB], BF16, isOutput=False)
    swapm = nc.declare_dram_parameter("swapmat", [128, 128], BF16, isOutput=False)
    out = nc.declare_dram_parameter("out", [NSLAB, SLAB // 4, HID], BF16, isOutput=True)
    if debug:
        dbg_qT = nc.declare_dram_parameter("dbg_qT", [NQ, 128, S], BF16,
                                           isOutput=True)
        dbg_kT = nc.declare_dram_parameter("dbg_kT", [NKV, 128, S], BF16,
                                           isOutput=True)
        dbg_v = nc.declare_dram_parameter("dbg_v", [S // 128, 128, 2 * D], BF16,
                                          isOutput=True)

    Exp = mybir.ActivationFunctionType.Exp

    with tile.TileContext(nc) as tc:
      with tc.tile_pool(name="dram", bufs=1, space="DRAM") as dram:
        qT_dram = [dram.tile([NQ, 128, CHUNK], BF16, name=f"qT_dram{c}",
                             tag=f"qT_dram{c}") for c in range(NCH)]
        parts = [dram.tile([SLAB, HID], BF16, name=f"part{s}", tag=f"part{s}")
                 for s in range(NSLAB)]
        rsouts = [dram.tile([SLAB // 4, HID], BF16, name=f"rsout{s}", tag=f"rsout{s}")
                  for s in range(NSLAB)]

        with tc.tile_pool(name="const", bufs=1) as cpool:
            cos_sb = cpool.tile([128, S], BF16, name="cos_sb", tag="cos_sb")
            sin_sb = cpool.tile([128, S], BF16, name="sin_sb", tag="sin_sb")
            ones_col = cpool.tile([128, 1], BF16, name="ones_col", tag="ones_col")
            nc.vector.memset(ones_col[:], 1.0)
            ones_row = cpool.tile([1, 128], F32, name="ones_row", tag="ones_row")
            nc.vector.memset(ones_row[:], 1.0)
            ones_row_bf = cpool.tile([1, 128], BF16, name="ones_row_bf",
                                     tag="ones_row_bf")
            nc.vector.memset(ones_row_bf[:], 1.0)
            swap_sb = cpool.tile([128, 128], BF16, name="swapm", tag="swapm")
            mask_sb = cpool.tile([128, SKT, SLAB], BF16, name="masks",
                                 tag="masks")

            # persistent across A0 -> A1
            with tc.tile_pool(name="qkv_keep", bufs=1) as kvp:
                kT_sb = [kvp.tile([128, S], BF16, name=f"kT{i}", tag=f"kT{i}")
                         for i in range(NKV)]
                v_sb = [kvp.tile([128, KC], BF16, name=f"v{t}", tag=f"v{t}")
                        for t in range(S // 128)]

                # ================= A0: QKV + RoPE =================
                with (
                    tc.tile_pool(name="wq", bufs=1) as wqp,
                    tc.tile_pool(name="hidT", bufs=2) as hTp,
                    tc.tile_pool(name="rope", bufs=2) as rp,
                    tc.tile_pool(name="psA", bufs=5, space="PSUM") as psA,
                    tc.tile_pool(name="psT", bufs=2, space="PSUM") as psT,
                ):
                    TGRP = 4   # h-tiles per transpose instruction (3D out)

                    def emit_transposes(c):
                        _lbl(f"A0.trans.c{c}")
                        groups = [hTp.tile([128, TGRP, CHUNK], BF16,
                                           name=f"hidTg{g}", tag=f"hidTg{g}")
                                  for g in range(NHT // TGRP)]
                        c0 = CHUNK * c
                        for g in range(NHT // TGRP):
                            # all transposes on ONE queue: two concurrent XBAR
                            # transposes (SP+Act) corrupt each other's 16-row
                            # tiles on hardware
                            nc.sync.dma_start_transpose(
                                groups[g][:],
                                hidden[c0:c0 + CHUNK,
                                       128 * TGRP * g:128 * TGRP * (g + 1)],
                            )
                        return groups

                    # startup: interleave wq-group loads (Act queue) with
                    # chunk-0 transpose groups (SP queue) in consumption order
                    # so DMA_ENGINES serves them h-ordered; consts at the end
                    _lbl("A0.wqload")
                    # weight load in 4 COLUMN blocks of 384 qkv-cols: block j
                    # feeds chunk-0 pass j (ct 3j..3j+2; block 3 also ct9+V).
                    # One block transfers in ~9us but feeds ~20us of PE work,
                    # so after block 0 lands the chunk-0 pipeline never stalls.
                    CB = (QC + 2 * KC) // 4
                    wq_b = [wqp.tile([128, NHT, CB], BF16,
                                     name=f"wqb{j}", tag=f"wqb{j}")
                            for j in range(4)]
                    hidT = emit_transposes(0)
                    for j in range(4):
                        nc.scalar.dma_start(
                            wq_b[j][:],
                            w_qkv[:, CB * j:CB * (j + 1)]
                            .rearrange("(h p) c -> p h c", p=128),
                        )
                    nc.sync.dma_start(cos_sb[:], cosd[:])
                    nc.sync.dma_start(sin_sb[:], sind[:])
                    nc.sync.dma_start(swap_sb[:], swapm[:])
                    nc.sync.dma_start(mask_sb[:], masks[:])

                    def wq_ap(h, lo, hi):
                        j = lo // CB
                        assert hi <= CB * (j + 1)
                        return wq_b[j][:, h, lo - CB * j:hi - CB * j]


                    for c in range(NCH):
                        c0 = CHUNK * c
                        # chunk 1's transposes are deferred to after chunk 0's
                        # ct loop: at startup the DMA sem lanes are saturated
                        # and they'd crowd out the wq group loads
                        hidT_next = (emit_transposes(c + 1)
                                     if 1 < c + 1 < NCH else None)
                        cosc = cos_sb[:, c0:c0 + CHUNK]
                        sinc = sin_sb[:, c0:c0 + CHUNK]
                        # Q^T / K^T col-tiles; RoPE pipelined one ct behind
                        # so pswap(ct) never stalls the PE behind qf(ct)
                        def emit_rope(ct, qf, c=c, c0=c0, cosc=cosc, sinc=sinc):
                            _lbl(f"A0.c{c}.rope{ct}")
                            pswap = psT.tile([128, CHUNK], F32, name="pswap",
                                             tag="pswap")
                            nc.tensor.matmul(pswap[:], swap_sb[:], qf[:],
                                             start=True, stop=True)
                            rot = rp.tile([128, CHUNK], BF16, name="rot", tag="rot")
                            nc.vector.tensor_mul(rot[:], pswap[:], sinc)
                            tc2 = rp.tile([128, CHUNK], BF16, name="tc2", tag="tc2")
                            nc.vector.tensor_mul(tc2[:], qf[:], cosc)
                            if ct < NQ:
                                qs = rp.tile([128, CHUNK], BF16, name="qs",
                                             tag="qs")
                                nc.vector.tensor_add(qs[:], tc2[:], rot[:])
                                nc.gpsimd.dma_start(
                                    qT_dram[c][ct, :, :], qs[:])
                            else:
                                nc.vector.tensor_add(
                                    kT_sb[ct - NQ][:, c0:c0 + CHUNK],
                                    tc2[:], rot[:])

                        pending = None
                        if c == 0:
                            # chunk 0: h-major in groups of <=3 ct chains so the
                            # PE consumes wq tiles as their DMAs land instead of
                            # stalling until the full weight load completes
                            groups = [(0, 1, 2), (3, 4, 5), (6, 7, 8), (9,)]
                            for grp in groups:
                                _lbl(f"A0.c0.grp{grp[0]}")
                                pq_map = {
                                    ct: psA.tile([128, CHUNK], F32,
                                                 name="pq", tag="pq")
                                    for ct in grp
                                }
                                for h in range(NHT):
                                    for ct in grp:
                                        nc.tensor.matmul(
                                            pq_map[ct][:],
                                            wq_ap(h, 128 * ct, 128 * (ct + 1)),
                                            hidT[h // TGRP][:, h % TGRP, :],
                                            start=(h == 0), stop=(h == NHT - 1),
                                        )
                                for ct in grp:
                                    qf = rp.tile([128, CHUNK], BF16,
                                                 name="qf", tag="qf")
                                    nc.scalar.copy(qf[:], pq_map[ct][:])
                                    if pending is not None:
                                        emit_rope(*pending)
                                    pending = (ct, qf)
                        else:
                            for ct in range(NQ + NKV):
                                _lbl(f"A0.c{c}.ct{ct}")
                                pq = psA.tile([128, CHUNK], F32,
                                              name="pq", tag="pq")
                                for h in range(NHT):
                                    nc.tensor.matmul(
                                        pq[:],
                                        wq_ap(h, 128 * ct, 128 * (ct + 1)),
                                        hidT[h // TGRP][:, h % TGRP, :],
                                        start=(h == 0), stop=(h == NHT - 1),
                                    )
                                qf = rp.tile([128, CHUNK], BF16,
                                             name="qf", tag="qf")
                                nc.scalar.copy(qf[:], pq[:])
                                if pending is not None:
                                    emit_rope(*pending)
                                pending = (ct, qf)
                        emit_rope(*pending)
                        if c == 0 and NCH > 1:
                            hidT_next = emit_transposes(1)
                        # V natural [tok, d]
                        for tt in range(KTC):
                            _lbl(f"A0.c{c}.v{tt}")
                            pv = psA.tile([128, KC], F32, name="pv", tag="pq")
                            for h in range(NHT):
                                nc.tensor.matmul(
                                    pv[:],
                                    hidT[h // TGRP][:, h % TGRP,
                                         128 * tt:128 * (tt + 1)],
                                    wq_ap(h, QC + KC, QC + 2 * KC),
                                    start=(h == 0), stop=(h == NHT - 1),
                                )
                            nc.scalar.copy(v_sb[c * KTC + tt][:], pv[:])
                        hidT = hidT_next

                if debug:
                    _lbl("debug.dump")
                    for c in range(NCH):
                        nc.sync.dma_start(
                            dbg_qT[:, :, CHUNK * c:CHUNK * (c + 1)],
                            qT_dram[c][:])
                    for i in range(NKV):
                        nc.sync.dma_start(dbg_kT[i], kT_sb[i][:])
                    for t in range(S // 128):
                        nc.sync.dma_start(dbg_v[t], v_sb[t][:])

                # ============ A1: attention + o-proj + RS ============
                with (
                    tc.tile_pool(name="wo", bufs=1) as wop,
                    tc.tile_pool(name="qsl", bufs=2) as qslp,
                    tc.tile_pool(name="at", bufs=2) as atp,
                    tc.tile_pool(name="pt", bufs=6) as ptp,
                    tc.tile_pool(name="den", bufs=2) as dnp,
                    tc.tile_pool(name="bcp", bufs=2) as bcp,
                    tc.tile_pool(name="ost", bufs=2) as ostp,
                    tc.tile_pool(name="psS", bufs=2, space="PSUM") as psS,
                    tc.tile_pool(name="psPV", bufs=2, space="PSUM") as psPV,
                    tc.tile_pool(name="psX", bufs=1, space="PSUM") as psX,
                    tc.tile_pool(name="psO", bufs=3, space="PSUM") as psO,
                ):
                    _lbl("A1.load")

                    def load_qsl(s):
                        # SLAB == CHUNK: slab s reads exactly chunk s's spill
                        qsl = qslp.tile([128, NQ, SLAB], BF16, name="qsl",
                                        tag="qsl")
                        nc.sync.dma_start(
                            qsl[:],
                            qT_dram[s][:].rearrange("h p c -> p h c"),
                        )
                        return qsl

                    qsl = load_qsl(0)
                    wo_sb = wop.tile([128, NQ, HID], BF16, name="wo", tag="wo")
                    CQ = HID // 4
                    for i in range(4):
                        eng = nc.sync if i % 2 == 0 else nc.scalar
                        eng.dma_start(
                            wo_sb[:, :, CQ * i:CQ * (i + 1)],
                            w_o[:, CQ * i:CQ * (i + 1)]
                            .rearrange("(q p) c -> p q c", p=128),
                        )
                    NHG = NHC // HG          # hg groups per tt (4)
                    NGRP = (SLAB // 128) * NHG  # o-proj groups per slab (16)

                    def make_oproj_emitter(s, at_tiles):
                        state = {}

                        def emit_groups(glo, ghi):
                            for g in range(glo, ghi):
                                tt, hg = divmod(g, NHG)
                                _lbl(f"A1.s{s}.oproj{tt}.{hg}")
                                if hg == 0:
                                    state["ost"] = ostp.tile(
                                        [128, HID], BF16, name="ost", tag="ost")
                                ost = state["ost"]
                                pos = [psO.tile([128, W], F32,
                                                name=f"po{j}", tag="po")
                                       for j in range(HG)]
                                for q in range(NQ):
                                    for j in range(HG):
                                        hc = hg * HG + j
                                        nc.tensor.matmul(
                                            pos[j][:],
                                            at_tiles[q][:, 128 * tt:128 * (tt + 1)],
                                            wo_sb[:, q, W * hc:W * (hc + 1)],
                                            start=(q == 0), stop=(q == NQ - 1),
                                        )
                                for j in range(HG):
                                    hc = hg * HG + j
                                    nc.vector.tensor_copy(
                                        ost[:, W * hc:W * (hc + 1)], pos[j][:])
                                if hg == NHG - 1:
                                    nc.sync.dma_start(
                                        parts[s][128 * tt:128 * (tt + 1), :],
                                        ost[:])
                        return emit_groups

                    def emit_rs(s, half):
                        _lbl(f"A1.s{s}.rs{half}")
                        r0 = (SLAB // 2) * half
                        o0 = (SLAB // 8) * half
                        nc.gpsimd.collective_compute(
                            "ReduceScatter",
                            mybir.AluOpType.add,
                            replica_groups=RG,
                            ins=[parts[s][r0:r0 + SLAB // 2, :]],
                            outs=[rsouts[s][o0:o0 + SLAB // 8, :]],
                        )
                        nc.gpsimd.dma_start(
                            out[s, o0:o0 + SLAB // 8, :],
                            rsouts[s][o0:o0 + SLAB // 8, :])

                    prev_emit = None  # o-proj emitter for slab s-1
                    for s in range(NSLAB):
                        s0 = SLAB * s
                        NKT = (s + 1) * SKT
                        qsl_next = load_qsl(s + 1) if s + 1 < NSLAB else None
                        at_tiles = []
                        for hq in range(NQ):
                            _lbl(f"A1.s{s}.h{hq}")
                            kvh = hq // REP
                            ppv = psPV.tile([128, SLAB], F32, name="ppv",
                                            tag="ppv")
                            den = dnp.tile([128, SLAB], BF16, name="den",
                                           tag="den")
                            # PV pipelined one kt behind: ps(kt+1) is emitted
                            # before ppv(kt) so the PE never waits on the exp
                            # round trip
                            pend_pv = None
                            for kt in range(NKT):
                                ps = psS.tile([128, SLAB], F32, name="ps",
                                              tag="ps")
                                nc.tensor.matmul(
                                    ps[:],
                                    kT_sb[kvh][:, 128 * kt:128 * (kt + 1)],
                                    qsl[:, hq, :],
                                    start=True, stop=True,
                                )
                                pt = ptp.tile([128, SLAB], BF16, name="pt",
                                              tag="pt")
                                nc.scalar.activation(pt[:], ps[:], Exp,
                                                     scale=SCALE)
                                diag = kt - s * SKT
                                if diag >= 0:
                                    nc.vector.tensor_mul(
                                        pt[:], pt[:], mask_sb[:, diag, :])
                                if kt == 0:
                                    nc.vector.tensor_copy(den[:], pt[:])
                                else:
                                    nc.vector.tensor_add(den[:], den[:], pt[:])
                                if pend_pv is not None:
                                    pkt, ppt = pend_pv
                                    nc.tensor.matmul(
                                        ppv[:],
                                        v_sb[pkt][:, D * kvh:D * (kvh + 1)],
                                        ppt[:],
                                        start=(pkt == 0), stop=False,
                                    )
                                pend_pv = (kt, pt)
                            pkt, ppt = pend_pv
                            nc.tensor.matmul(
                                ppv[:],
                                v_sb[pkt][:, D * kvh:D * (kvh + 1)],
                                ppt[:],
                                start=(pkt == 0), stop=True,
                            )
                            pden = psX.tile([128, SLAB], F32, name="pden",
                                            tag="pden")
                            nc.tensor.matmul(pden[0:1, :], ones_col[:], den[:],
                                             start=True, stop=True)
                            rec = dnp.tile([1, SLAB], F32, name="rec", tag="rec")
                            nc.vector.reciprocal_approx_fast(rec[:], pden[0:1, :])
                            recb = dnp.tile([1, SLAB], BF16, name="recb",
                                            tag="recb")
                            nc.vector.tensor_copy(recb[:], rec[:])
                            pbc = psX.tile([128, SLAB], F32, name="pbc",
                                           tag="pden")
                            nc.tensor.matmul(pbc[:], ones_row_bf[:], recb[:],
                                             start=True, stop=True)
                            bc = bcp.tile([128, SLAB], F32, name="bc", tag="bc")
                            nc.vector.tensor_copy(bc[:], pbc[:])
                            at = atp.tile([128, SLAB], BF16,
                                          name=f"at{hq}", tag=f"at{hq}")
                            nc.vector.tensor_mul(at[:], ppv[:], bc[:])
                            at_tiles.append(at)
                            # interleave previous slab's o-proj between heads
                            if prev_emit is not None:
                                prev_emit(2 * hq, 2 * hq + 2)
                                if hq == 3:
                                    emit_rs(s - 1, 0)
                                elif hq == 7:
                                    emit_rs(s - 1, 1)
                        prev_emit = make_oproj_emitter(s, at_tiles)
                        qsl = qsl_next
                    # final slab's o-proj + RS
                    prev_emit(0, NGRP // 2)
                    emit_rs(NSLAB - 1, 0)
                    prev_emit(NGRP // 2, NGRP)
                    emit_rs(NSLAB - 1, 1)

    nc.compile()
    return nc


def make_consts(S=2048):
    SLAB = min(512, S)
    SKT = SLAB // 128
    p = np.arange(128).reshape(128, 1, 1)
    j = np.arange(SKT).reshape(1, SKT, 1)
    q = np.arange(SLAB).reshape(1, 1, SLAB)
    masks = ((j * 128 + p) <= q).astype(ml_dtypes.bfloat16)  # [128, SKT, SLAB]
    # swap matrix with RoPE sign baked in: out[m<64] = -x[m+64], out[m>=64] = +x[m-64]
    swapmat = np.zeros((128, 128), np.float32)
    for m in range(128):
        swapmat[(m + 64) % 128, m] = 1.0 if m >= 64 else -1.0
    return masks, swapmat.astype(ml_dtypes.bfloat16)


def host_trig(positions_row):
    """cos/sin [128, S] bf16 for one batch's positions (RoPE angles)."""
    d_half = np.arange(0, D, 2, dtype=np.float64) / D
    invf = 1.0 / (ROPE_THETA ** d_half)                 # [64]
    invf128 = np.concatenate([invf, invf])              # [128]
    ang = invf128[:, None] * positions_row[None, :].astype(np.float64)
    return (np.cos(ang).astype(ml_dtypes.bfloat16),
            np.sin(ang).astype(ml_dtypes.bfloat16))


def shard_inputs(hidden_states, positions, w_qkv, w_o, n_q_total=32, n_kv_total=8,
                 tp=4):
    """Returns in_maps for 8 cores: DP over batch x TP over heads."""
    B, S, HIDDEN = hidden_states.shape
    q_size = n_q_total * D
    kv_size = n_kv_total * D
    nq = n_q_total // tp           # q heads per core
    nkv = n_kv_total // tp         # kv heads per core
    masks, swapmat = make_consts(S=S)
    hid_bf = hidden_states.astype(ml_dtypes.bfloat16)
    trigs = [host_trig(np.asarray(positions[g]).astype(np.int64))
             for g in range(B)]
    wq_bf = w_qkv.astype(ml_dtypes.bfloat16)
    wo_bf = w_o.astype(ml_dtypes.bfloat16)
    in_maps = []
    for c in range(8):
        g, r = divmod(c, tp)
        wq = wq_bf[:, nq * D * r: nq * D * (r + 1)]
        wk = wq_bf[:, q_size + nkv * D * r: q_size + nkv * D * (r + 1)]
        wv = wq_bf[:, q_size + kv_size + nkv * D * r:
                   q_size + kv_size + nkv * D * (r + 1)]
        in_maps.append({
            "hidden_states": np.ascontiguousarray(hid_bf[g]),
            "w_qkv": np.ascontiguousarray(np.concatenate([wq, wk, wv], axis=1)),
            "w_o": np.ascontiguousarray(wo_bf[nq * D * r: nq * D * (r + 1), :]),
            "cos_all": trigs[g][0],
            "sin_all": trigs[g][1],
            "masks": masks,
            "swapmat": swapmat,
        })
    return in_maps


def assemble_output(results, B=2, S=2048, HIDDEN=4096, tp=4):
    SLAB = min(512, S)
    NSLAB = S // SLAB
    RSROWS = SLAB // 4   # rows per core per slab (128)
    RSH = SLAB // 8      # rows per core per half-RS (64)
    out = np.empty((B, S, HIDDEN), dtype=np.float32)
    for c in range(8):
        g, r = divmod(c, tp)
        o = np.asarray(results[c]["out"]).astype(np.float32).reshape(
            NSLAB, RSROWS, HIDDEN)
        for s in range(NSLAB):
            for h in range(2):
                t0 = SLAB * s + (SLAB // 2) * h + RSH * r
                out[g, t0:t0 + RSH, :] = o[s, RSH * h:RSH * (h + 1), :]
    return out


def kernel(hidden_states, positions, w_qkv, w_o):
    hidden_states = np.asarray(hidden_states, dtype=np.float32)
    positions = np.asarray(positions, dtype=np.int32)
    w_qkv = np.asarray(w_qkv, dtype=np.float32)
    w_o = np.asarray(w_o, dtype=np.float32)
    B, S, HIDDEN = hidden_states.shape

    key = (S, HIDDEN)
    if key not in _NC_CACHE:
        _NC_CACHE[key] = build_nc(S=S, HID=HIDDEN)
    nc = _NC_CACHE[key]

    in_maps = shard_inputs(hidden_states, positions, w_qkv, w_o)
    res = run_bass_kernel_spmd(nc, in_maps, core_ids=list(range(8)))
    return assemble_output(res.results, B=B, S=S, HIDDEN=HIDDEN)


if __name__ == "__main__":
    rng = np.random.default_rng(0)
    B, S, HIDDEN = 2, 2048, 4096
    hs = rng.standard_normal((B, S, HIDDEN), dtype=np.float32)
    pos = np.arange(B * S, dtype=np.int32).reshape(B, S)
    wq = rng.standard_normal((HIDDEN, 6144), dtype=np.float32) * HIDDEN ** -0.5
    wo = rng.standard_normal((4096, HIDDEN), dtype=np.float32) * 4096 ** -0.5
    o = kernel(hs, pos, wq, wo)
    print(o.shape, o.dtype)


# revision 59
# speedup vs baseline: 1.0962x; 1.0326x over previous
"""Distributed Trainium2 kernel for ArceeAttention (GQA + RoPE + causal attention).

Sharding: DP over batch (2 groups of 4 cores) x TP-4 over heads within each
group. Each core: 8 q heads + 2 kv heads, full sequence of its batch.
ReduceScatter(add) over each 4-core group combines o-proj partials per
512-token slab, split into two half-RS so communication overlaps compute.

Design (v2, ~834us in TimelineSim vs 1045us baseline; HW-verified 7.0e-3):
  - Host side: hidden/w_qkv/w_o cast to bf16, RoPE cos/sin precomputed as
    bf16 [128, S] tables, causal masks + sign-baked rotate-half swap matrix.
    No SWDGE casts, no on-device trig, no DRAM staging round trips.
  - A0 (QKV+RoPE), CHUNK=512: hidden transposed straight from DRAM via XBAR
    in [512,512]->[128,4,512] grouped transposes, ALL on the SP queue (two
    concurrent XBAR transposes on different HWDGE queues corrupt each other
    on HW). Weights loaded as 4 COLUMN-block DMAs on the Act queue; chunk 0
    runs h-major in per-block groups of <=3 ct chains, so each ~9us block
    transfer feeds ~20us of PE work and the pipeline never starves after
    block 0 lands. RoPE is pipelined one ct behind its QKV chain;
    rotate-half via a sign-baked bf16 swap matmul.
  - qT spilled to per-chunk DRAM tiles (frees SBUF for the 96KB weight
    tile), reloaded per slab in A1 with one strided DMA; kT/V stay resident.
  - A1: causal attention with 512-wide transposed score blocks, exp without
    max-subtraction, denominator via ones-matmul + fast reciprocal; PV
    accumulation pipelined one kt behind the scores/exp. O-proj of slab s-1
    is interleaved between attention heads of slab s so the PE fills the
    Act-engine exp latency; outputs staged as [128, HID] rows (4 DMAs/slab).
  - Output dtype bf16 on device; host assembles/casts to f32.
"""
import sys
import numpy as np

for _p in ("/opt/trn_rl_repo",):
    if _p not in sys.path:
        sys.path.append(_p)

import ml_dtypes  # noqa: E402
from concourse import bass, bacc, tile, mybir  # noqa: E402
from concourse.bass_utils import run_bass_kernel_spmd  # noqa: E402

F32 = mybir.dt.float32
BF16 = mybir.dt.bfloat16
I32 = mybir.dt.int32

ROPE_THETA = 10000.0
D = 128  # head dim

_NC_CACHE = {}
TRACE_HOOK = None


def _lbl(s):
    if TRACE_HOOK:
        TRACE_HOOK(s)


def build_nc(S=2048, HID=4096, NQ=8, NKV=2, CHUNK=512, debug=False):
    REP = NQ // NKV
    QC = NQ * D               # q cols per core
    KC = NKV * D              # k (or v) cols per core
    NCH = S // CHUNK          # token chunks in A0
    KTC = CHUNK // 128
    NHT = HID // 128          # hidden-dim tiles
    SLAB = min(512, S)        # tokens per ReduceScatter slab / query block
    NSLAB = S // SLAB
    SKT = SLAB // 128         # k-tiles per slab
    NHC = HID // 512          # o-proj col chunks
    W = 512
    HG = 2                    # o-proj psum group size
    SCALE = float(D) ** -0.5
    RG = [[0, 1, 2, 3], [4, 5, 6, 7]]

    MAGIC = 12582912.0        # 1.5 * 2**23: float32 round-to-nearest-int trick
    TWOPI = float(2.0 * np.pi)
    INV2PI = float(1.0 / TWOPI)
    HALFPI = float(np.pi / 2.0)

    nc = bacc.Bacc(None, target_bir_lowering=False)
    hidden = nc.declare_dram_parameter("hidden_states", [S, HID], BF16, isOutput=False)
    w_qkv = nc.declare_dram_parameter("w_qkv", [HID, QC + 2 * KC], BF16, isOutput=False)
    w_o = nc.declare_dram_parameter("w_o", [QC, HID], BF16, isOutput=False)
    cosd = nc.declare_dram_parameter("cos_all", [128, S], BF16, isOutput=False)
    sind = nc.declare_dram_parameter("sin_all", [128, S], BF16, isOutput=False)
    masks = nc.declare_dram_parameter("masks", [128, SKT, SLAB], BF16, isOutput=False)
    swapm = nc.declare_dram_parameter("swapmat", [128, 128], BF16, isOutput=False)
    out = nc.declare_dram_parameter("out", [NSLAB, SLAB // 4, HID], BF16, isOutput=True)
    if debug:
        dbg_qT = nc.declare_dram_parameter("dbg_qT", [NQ, 128, S], BF16,
                                           isOutput=True)
        dbg_kT = nc.declare_dram_parameter("dbg_kT", [NKV, 128, S], BF16,
                                           isOutput=True)
        dbg_v = nc.declare_dram_parameter("dbg_v", [S // 128, 128, 2 * D], BF16,
                                          isOutput=True)

    Exp = mybir.ActivationFunctionType.Exp

    with tile.TileContext(nc) as tc:
      with tc.tile_pool(name="dram", bufs=1, space="DRAM") as dram:
        qT_dram = [dram.tile([NQ, 128, CHUNK], BF16, name=f"qT_dram{c}",
                             tag=f"qT_dram{c}") for c in range(NCH)]
        parts = [dram.tile([SLAB, HID], BF16, name=f"part{s}", tag=f"part{s}")
                 for s in range(NSLAB)]
        rsouts = [dram.tile([SLAB // 4, HID], BF16, name=f"rsout{s}", tag=f"rsout{s}")
                  for s in range(NSLAB)]

        with tc.tile_pool(name="const", bufs=1) as cpool:
            cos_sb = cpool.tile([128, S], BF16, name="cos_sb", tag="cos_sb")
            sin_sb = cpool.tile([128, S], BF16, name="sin_sb", tag="sin_sb")
            ones_col = cpool.tile([128, 1], BF16, name="ones_col", tag="ones_col")
            nc.vector.memset(ones_col[:], 1.0)
            ones_row = cpool.tile([1, 128], F32, name="ones_row", tag="ones_row")
            nc.vector.memset(ones_row[:], 1.0)
            ones_row_bf = cpool.tile([1, 128], BF16, name="ones_row_bf",
                                     tag="ones_row_bf")
            nc.vector.memset(ones_row_bf[:], 1.0)
            swap_sb = cpool.tile([128, 128], BF16, name="swapm", tag="swapm")
            mask_sb = cpool.tile([128, SKT, SLAB], BF16, name="masks",
                                 tag="masks")

            # persistent across A0 -> A1
            with tc.tile_pool(name="qkv_keep", bufs=1) as kvp:
                kT_sb = [kvp.tile([128, S], BF16, name=f"kT{i}", tag=f"kT{i}")
                         for i in range(NKV)]
                v_sb = [kvp.tile([128, KC], BF16, name=f"v{t}", tag=f"v{t}")
                        for t in range(S // 128)]

                # ================= A0: QKV + RoPE =================
                with (
                    tc.tile_pool(name="wq", bufs=1) as wqp,
                    tc.tile_pool(name="hidT", bufs=2) as hTp,
                    tc.tile_pool(name="rope", bufs=2) as rp,
                    tc.tile_pool(name="psA", bufs=5, space="PSUM") as psA,
                    tc.tile_pool(name="psT", bufs=2, space="PSUM") as psT,
                ):
                    TGRP = 4   # h-tiles per transpose instruction (3D out)

                    def emit_transposes(c):
                        _lbl(f"A0.trans.c{c}")
                        groups = [hTp.tile([128, TGRP, CHUNK], BF16,
                                           name=f"hidTg{g}", tag=f"hidTg{g}")
                                  for g in range(NHT // TGRP)]
                        c0 = CHUNK * c
                        for g in range(NHT // TGRP):
                            # all transposes on ONE queue: two concurrent XBAR
                            # transposes (SP+Act) corrupt each other's 16-row
                            # tiles on hardware
                            nc.sync.dma_start_transpose(
                                groups[g][:],
                                hidden[c0:c0 + CHUNK,
                                       128 * TGRP * g:128 * TGRP * (g + 1)],
                            )
                        return groups

                    # startup: interleave wq-group loads (Act queue) with
                    # chunk-0 transpose groups (SP queue) in consumption order
                    # so DMA_ENGINES serves them h-ordered; consts at the end
                    _lbl("A0.wqload")
                    # weight load in 4 COLUMN blocks of 384 qkv-cols: block j
                    # feeds chunk-0 pass j (ct 3j..3j+2; block 3 also ct9+V).
                    # One block transfers in ~9us but feeds ~20us of PE work,
                    # so after block 0 lands the chunk-0 pipeline never stalls.
                    CB = (QC + 2 * KC) // 4
                    wq_b = [wqp.tile([128, NHT, CB], BF16,
                                     name=f"wqb{j}", tag=f"wqb{j}")
                            for j in range(4)]
                    hidT = emit_transposes(0)
                    for j in range(4):
                        nc.scalar.dma_start(
                            wq_b[j][:],
                            w_qkv[:, CB * j:CB * (j + 1)]
                            .rearrange("(h p) c -> p h c", p=128),
                        )
                    nc.sync.dma_start(cos_sb[:], cosd[:])
                    nc.sync.dma_start(sin_sb[:], sind[:])
                    nc.sync.dma_start(swap_sb[:], swapm[:])
                    nc.sync.dma_start(mask_sb[:], masks[:])

                    def wq_ap(h, lo, hi):
                        j = lo // CB
                        assert hi <= CB * (j + 1)
                        return wq_b[j][:, h, lo - CB * j:hi - CB * j]


                    for c in range(NCH):
                        c0 = CHUNK * c
                        # chunk 1's transposes are deferred to after chunk 0's
                        # ct loop: at startup the DMA sem lanes are saturated
                        # and they'd crowd out the wq group loads
                        hidT_next = (emit_transposes(c + 1)
                                     if 1 < c + 1 < NCH else None)
                        cosc = cos_sb[:, c0:c0 + CHUNK]
                        sinc = sin_sb[:, c0:c0 + CHUNK]
                        # Q^T / K^T col-tiles; RoPE pipelined one ct behind
                        # so pswap(ct) never stalls the PE behind qf(ct)
                        def emit_rope(ct, qf, c=c, c0=c0, cosc=cosc, sinc=sinc):
                            _lbl(f"A0.c{c}.rope{ct}")
                            pswap = psT.tile([128, CHUNK], F32, name="pswap",
                                             tag="pswap")
                            nc.tensor.matmul(pswap[:], swap_sb[:], qf[:],
                                             start=True, stop=True)
                            rot = rp.tile([128, CHUNK], BF16, name="rot", tag="rot")
                            nc.vector.tensor_mul(rot[:], pswap[:], sinc)
                            tc2 = rp.tile([128, CHUNK], BF16, name="tc2", tag="tc2")
                            nc.vector.tensor_mul(tc2[:], qf[:], cosc)
                            if ct < NQ:
                                qs = rp.tile([128, CHUNK], BF16, name="qs",
                                             tag="qs")
                                nc.vector.tensor_add(qs[:], tc2[:], rot[:])
                                nc.gpsimd.dma_start(
                                    qT_dram[c][ct, :, :], qs[:])
                            else:
                                nc.vector.tensor_add(
                                    kT_sb[ct - NQ][:, c0:c0 + CHUNK],
                                    tc2[:], rot[:])

                        pending = None
                        if c == 0:
                            # chunk 0: h-major in groups of <=3 ct chains so the
                            # PE consumes wq tiles as their DMAs land instead of
                            # stalling until the full weight load completes
                            groups = [(0, 1, 2), (3, 4, 5), (6, 7, 8), (9,)]
                            for grp in groups:
                                _lbl(f"A0.c0.grp{grp[0]}")
                                pq_map = {
                                    ct: psA.tile([128, CHUNK], F32,
                                                 name="pq", tag="pq")
                                    for ct in grp
                                }
                                for h in range(NHT):
                                    for ct in grp:
                                        nc.tensor.matmul(
                                            pq_map[ct][:],
                                            wq_ap(h, 128 * ct, 128 * (ct + 1)),
                                            hidT[h // TGRP][:, h % TGRP, :],
                                            start=(h == 0), stop=(h == NHT - 1),
                                        )
                                for ct in grp:
                                    qf = rp.tile([128, CHUNK], BF16,
                                                 name="qf", tag="qf")
                                    nc.scalar.copy(qf[:], pq_map[ct][:])
                                    if pending is not None:
                                        emit_rope(*pending)
                                    pending = (ct, qf)
                        else:
                            for ct in range(NQ + NKV):
                                _lbl(f"A0.c{c}.ct{ct}")
                                pq = psA.tile([128, CHUNK], F32,
                                              name="pq", tag="pq")
                                for h in range(NHT):
                                    nc.tensor.matmul(
                                        pq[:],
                                        wq_ap(h, 128 * ct, 128 * (ct + 1)),
                                        hidT[h // TGRP][:, h % TGRP, :],
                                        start=(h == 0), stop=(h == NHT - 1),
                                    )
                                qf = rp.tile([128, CHUNK], BF16,
                                             name="qf", tag="qf")
                                nc.scalar.copy(qf[:], pq[:])
                                if pending is not None:
                                    emit_rope(*pending)
                                pending = (ct, qf)
                        emit_rope(*pending)
                        if c == 0 and NCH > 1:
                            hidT_next = emit_transposes(1)
                        # V natural [tok, d]
                        for tt in range(KTC):
                            _lbl(f"A0.c{c}.v{tt}")
                            pv = psA.tile([128, KC], F32, name="pv", tag="pq")
                            for h in range(NHT):
                                nc.tensor.matmul(
                                    pv[:],
                                    hidT[h // TGRP][:, h % TGRP,
                                         128 * tt:128 * (tt + 1)],
                                    wq_ap(h, QC + KC, QC + 2 * KC),
                                    start=(h == 0), stop=(h == NHT - 1),
                                )
                            nc.scalar.copy(v_sb[c * KTC + tt][:], pv[:])
                        hidT = hidT_next

                if debug:
                    _lbl("debug.dump")
                    for c in range(NCH):
                        nc.sync.dma_start(
                            dbg_qT[:, :, CHUNK * c:CHUNK * (c + 1)],
                            qT_dram[c][:])
                    for i in range(NKV):
                        nc.sync.dma_start(dbg_kT[i], kT_sb[i][:])
                    for t in range(S // 128):
                        nc.sync.dma_start(dbg_v[t], v_sb[t][:])

                # ============ A1: attention + o-proj + RS ============
                with (
                    tc.tile_pool(name="wo", bufs=1) as wop,
                    tc.tile_pool(name="qsl", bufs=2) as qslp,
                    tc.tile_pool(name="at", bufs=2) as atp,
                    tc.tile_pool(name="pt", bufs=6) as ptp,
                    tc.tile_pool(name="den", bufs=2) as dnp,
                    tc.tile_pool(name="bcp", bufs=2) as bcp,
                    tc.tile_pool(name="ost", bufs=2) as ostp,
                    tc.tile_pool(name="psS", bufs=2, space="PSUM") as psS,
                    tc.tile_pool(name="psPV", bufs=2, space="PSUM") as psPV,
                    tc.tile_pool(name="psX", bufs=1, space="PSUM") as psX,
                    tc.tile_pool(name="psO", bufs=3, space="PSUM") as psO,
                ):
                    _lbl("A1.load")

                    def load_qsl(s):
                        # SLAB == CHUNK: slab s reads exactly chunk s's spill
                        qsl = qslp.tile([128, NQ, SLAB], BF16, name="qsl",
                                        tag="qsl")
                        nc.sync.dma_start(
                            qsl[:],
                            qT_dram[s][:].rearrange("h p c -> p h c"),
                        )
                        return qsl

                    qsl = load_qsl(0)
                    wo_sb = wop.tile([128, NQ, HID], BF16, name="wo", tag="wo")
                    CQ = HID // 4
                    for i in range(4):
                        eng = nc.sync if i % 2 == 0 else nc.scalar
                        eng.dma_start(
                            wo_sb[:, :, CQ * i:CQ * (i + 1)],
                            w_o[:, CQ * i:CQ * (i + 1)]
                            .rearrange("(q p) c -> p q c", p=128),
                        )
                    NHG = NHC // HG          # hg groups per tt (4)
                    NGRP = (SLAB // 128) * NHG  # o-proj groups per slab (16)

                    def make_oproj_emitter(s, at_tiles):
                        state = {}

                        def emit_groups(glo, ghi):
                            for g in range(glo, ghi):
                                tt, hg = divmod(g, NHG)
                                _lbl(f"A1.s{s}.oproj{tt}.{hg}")
                                if hg == 0:
                                    state["ost"] = ostp.tile(
                                        [128, HID], BF16, name="ost", tag="ost")
                                ost = state["ost"]
                                pos = [psO.tile([128, W], F32,
                                                name=f"po{j}", tag="po")
                                       for j in range(HG)]
                                for q in range(NQ):
                                    for j in range(HG):
                                        hc = hg * HG + j
                                        nc.tensor.matmul(
                                            pos[j][:],
                                            at_tiles[q][:, 128 * tt:128 * (tt + 1)],
                                            wo_sb[:, q, W * hc:W * (hc + 1)],
                                            start=(q == 0), stop=(q == NQ - 1),
                                        )
                                for j in range(HG):
                                    hc = hg * HG + j
                                    nc.vector.tensor_copy(
                                        ost[:, W * hc:W * (hc + 1)], pos[j][:])
                                if hg == NHG - 1:
                                    nc.sync.dma_start(
                                        parts[s][128 * tt:128 * (tt + 1), :],
                                        ost[:])
                        return emit_groups

                    def emit_rs(s, half):
                        _lbl(f"A1.s{s}.rs{half}")
                        r0 = (SLAB // 2) * half
                        o0 = (SLAB // 8) * half
                        nc.gpsimd.collective_compute(
                            "ReduceScatter",
                            mybir.AluOpType.add,
                            replica_groups=RG,
                            ins=[parts[s][r0:r0 + SLAB // 2, :]],
                            outs=[rsouts[s][o0:o0 + SLAB // 8, :]],
                        )
                        nc.gpsimd.dma_start(
                            out[s, o0:o0 + SLAB // 8, :],
                            rsouts[s][o0:o0 + SLAB // 8, :])

                    prev_emit = None  # o-proj emitter for slab s-1
                    for s in range(NSLAB):
                        s0 = SLAB * s
                        NKT = (s + 1) * SKT
                        qsl_next = load_qsl(s + 1) if s + 1 < NSLAB else None
                        at_tiles = []
                        for hq in range(NQ):
                            _lbl(f"A1.s{s}.h{hq}")
                            kvh = hq // REP
                            ppv = psPV.tile([128, SLAB], F32, name="ppv",
                                            tag="ppv")
                            den = dnp.tile([128, SLAB], BF16, name="den",
                                           tag="den")
                            # PV pipelined one kt behind: ps(kt+1) is emitted
                            # before ppv(kt) so the PE never waits on the exp
                            # round trip
                            pend_pv = None
                            for kt in range(NKT):
                                ps = psS.tile([128, SLAB], F32, name="ps",
                                              tag="ps")
                                nc.tensor.matmul(
                                    ps[:],
                                    kT_sb[kvh][:, 128 * kt:128 * (kt + 1)],
                                    qsl[:, hq, :],
                                    start=True, stop=True,
                                )
                                pt = ptp.tile([128, SLAB], BF16, name="pt",
                                              tag="pt")
                                nc.scalar.activation(pt[:], ps[:], Exp,
                                                     scale=SCALE)
                                diag = kt - s * SKT
                                if diag >= 0:
                                    nc.vector.tensor_mul(
                                        pt[:], pt[:], mask_sb[:, diag, :])
                                if kt == 0:
                                    nc.vector.tensor_copy(den[:], pt[:])
                                else:
                                    nc.vector.tensor_add(den[:], den[:], pt[:])
                                if pend_pv is not None:
                                    pkt, ppt = pend_pv
                                    nc.tensor.matmul(
                                        ppv[:],
                                        v_sb[pkt][:, D * kvh:D * (kvh + 1)],
                                        ppt[:],
                                        start=(pkt == 0), stop=False,
                                    )
                                pend_pv = (kt, pt)
                            pkt, ppt = pend_pv
                            nc.tensor.matmul(
                                ppv[:],
                                v_sb[pkt][:, D * kvh:D * (kvh + 1)],
                                ppt[:],
                                start=(pkt == 0), stop=True,
                            )
                            pden = psX.tile([128, SLAB], F32, name="pden",
                                            tag="pden")
                            nc.tensor.matmul(pden[0:1, :], ones_col[:], den[:],
                                             start=True, stop=True)
                            rec = dnp.tile([1, SLAB], F32, name="rec", tag="rec")
                            nc.vector.reciprocal_approx_fast(rec[:], pden[0:1, :])
                            recb = dnp.tile([1, SLAB], BF16, name="recb",
                                            tag="recb")
                            nc.vector.tensor_copy(recb[:], rec[:])
                            pbc = psX.tile([128, SLAB], F32, name="pbc",
                                           tag="pden")
                            nc.tensor.matmul(pbc[:], ones_row_bf[:], recb[:],
                                             start=True, stop=True)
                            bc = bcp.tile([128, SLAB], F32, name="bc", tag="bc")
                            nc.vector.tensor_copy(bc[:], pbc[:])
                            at = atp.tile([128, SLAB], BF16,
                                          name=f"at{hq}", tag=f"at{hq}")
                            nc.vector.tensor_mul(at[:], ppv[:], bc[:])
                            at_tiles.append(at)
                            # interleave previous slab's o-proj between heads
                            if prev_emit is not None:
                                prev_emit(2 * hq, 2 * hq + 2)
                                if hq == 3:
                                    emit_rs(s - 1, 0)
                                elif hq == 7:
                                    emit_rs(s - 1, 1)
                        prev_emit = make_oproj_emitter(s, at_tiles)
                        qsl = qsl_next
                    # final slab's o-proj + RS
                    prev_emit(0, NGRP // 2)
                    emit_rs(NSLAB - 1, 0)
                    prev_emit(NGRP // 2, NGRP)
                    emit_rs(NSLAB - 1, 1)

    nc.compile()
    return nc


def make_consts(S=2048):
    SLAB = min(512, S)
    SKT = SLAB // 128
    p = np.arange(128).reshape(128, 1, 1)
    j = np.arange(SKT).reshape(1, SKT, 1)
    q = np.arange(SLAB).reshape(1, 1, SLAB)
    masks = ((j * 128 + p) <= q).astype(ml_dtypes.bfloat16)  # [128, SKT, SLAB]
    # swap matrix with RoPE sign baked in: out[m<64] = -x[m+64], out[m>=64] = +x[m-64]
    swapmat = np.zeros((128, 128), np.float32)
    for m in range(128):
        swapmat[(m + 64) % 128, m] = 1.0 if m >= 64 else -1.0
    return masks, swapmat.astype(ml_dtypes.bfloat16)


def host_trig(positions_row):
    """cos/sin [128, S] bf16 for one batch's positions (RoPE angles)."""
    d_half = np.arange(0, D, 2, dtype=np.float64) / D
    invf = 1.0 / (ROPE_THETA ** d_half)                 # [64]
    invf128 = np.concatenate([invf, invf])              # [128]
    ang = invf128[:, None] * positions_row[None, :].astype(np.float64)
    return (np.cos(ang).astype(ml_dtypes.bfloat16),
            np.sin(ang).astype(ml_dtypes.bfloat16))


def shard_inputs(hidden_states, positions, w_qkv, w_o, n_q_total=32, n_kv_total=8,
                 tp=4):
    """Returns in_maps for 8 cores: DP over batch x TP over heads."""
    B, S, HIDDEN = hidden_states.shape
    q_size = n_q_total * D
    kv_size = n_kv_total * D
    nq = n_q_total // tp           # q heads per core
    nkv = n_kv_total // tp         # kv heads per core
    masks, swapmat = make_consts(S=S)
    hid_bf = hidden_states.astype(ml_dtypes.bfloat16)
    trigs = [host_trig(np.asarray(positions[g]).astype(np.int64))
             for g in range(B)]
    wq_bf = w_qkv.astype(ml_dtypes.bfloat16)
    wo_bf = w_o.astype(ml_dtypes.bfloat16)
    in_maps = []
    for c in range(8):
        g, r = divmod(c, tp)
        wq = wq_bf[:, nq * D * r: nq * D * (r + 1)]
        wk = wq_bf[:, q_size + nkv * D * r: q_size + nkv * D * (r + 1)]
        wv = wq_bf[:, q_size + kv_size + nkv * D * r:
                   q_size + kv_size + nkv * D * (r + 1)]
        in_maps.append({
            "hidden_states": np.ascontiguousarray(hid_bf[g]),
            "w_qkv": np.ascontiguousarray(np.concatenate([wq, wk, wv], axis=1)),
            "w_o": np.ascontiguousarray(wo_bf[nq * D * r: nq * D * (r + 1), :]),
            "cos_all": trigs[g][0],
            "sin_all": trigs[g][1],
            "masks": masks,
            "swapmat": swapmat,
        })
    return in_maps


def assemble_output(results, B=2, S=2048, HIDDEN=4096, tp=4):
    SLAB = min(512, S)
    NSLAB = S // SLAB
    RSROWS = SLAB // 4   # rows per core per slab (128)
    RSH = SLAB // 8      # rows per core per half-RS (64)
    out = np.empty((B, S, HIDDEN), dtype=np.float32)
    for c in range(8):
        g, r = divmod(c, tp)
        o = np.asarray(results[c]["out"]).astype(np.float32).reshape(
            NSLAB, RSROWS, HIDDEN)
        for s in range(NSLAB):
            for h in range(2):
                t0 = SLAB * s + (SLAB // 2) * h + RSH * r
                out[g, t0:t0 + RSH, :] = o[s, RSH * h:RSH * (h + 1), :]
    return out


def kernel(hidden_states, positions, w_qkv, w_o):
    hidden_states = np.asarray(hidden_states, dtype=np.float32)
    positions = np.asarray(positions, dtype=np.int32)
    w_qkv = np.asarray(w_qkv, dtype=np.float32)
    w_o = np.asarray(w_o, dtype=np.float32)
    B, S, HIDDEN = hidden_states.shape

    key = (S, HIDDEN)
    if key not in _NC_CACHE:
        _NC_CACHE[key] = build_nc(S=S, HID=HIDDEN)
    nc = _NC_CACHE[key]

    in_maps = shard_inputs(hidden_states, positions, w_qkv, w_o)
    res = run_bass_kernel_spmd(nc, in_maps, core_ids=list(range(8)))
    return assemble_output(res.results, B=B, S=S, HIDDEN=HIDDEN)


if __name__ == "__main__":
    rng = np.random.default_rng(0)
    B, S, HIDDEN = 2, 2048, 4096
    hs = rng.standard_normal((B, S, HIDDEN), dtype=np.float32)
    pos = np.arange(B * S, dtype=np.int32).reshape(B, S)
    wq = rng.standard_normal((HIDDEN, 6144), dtype=np.float32) * HIDDEN ** -0.5
    wo = rng.standard_normal((4096, HIDDEN), dtype=np.float32) * 4096 ** -0.5
    o = kernel(hs, pos, wq, wo)
    print(o.shape, o.dtype)
